# revision 1
# baseline (speedup 1.0000x reference)
"""Causal multi-head attention (B=2, S=2048, D=1024, H=16) on 8 trn2 cores.

Sharding: core c handles heads {2c, 2c+1} of BOTH batches (4 (b,h) pairs).
Per core:
  - project host-pretransposed x_b^T [D, S] (both batches) through the
    core's Wqkv column slice into Q^T/K^T head-pair tiles and V (natural
    layout, with a fused ones-column that makes the AV matmul emit softmax
    denominators),
  - causal attention per (batch, head) in transposed layout: scores^T =
    K Q^T chunks (PE row-tiled head pairs), exp on ScalarE, causal diagonal
    masks via gpsimd affine_select, A^T V on PE,
  - one 8-wide AllToAll redistributes head outputs so core c holds ALL 16
    heads of batch c//4 for sequence quarter c%4,
  - local projection through the full Wout emits final rows
    512*(c%4) .. +512 of batch c//4.
Host assembles the 8 [512, 1024] shards into (2, 2048, 1024).

Matmuls run in float32r (TF32-like single-pass PE mode, ~1e-3 rel err,
4x faster than true fp32). The PE rounds f32r inputs internally, so DRAM
inputs are declared float32r and DMA'd with the fast HW-DGE path with no
pre-rounding. Set _USE_F32R = False for full fp32.
"""

import sys

for _p in ("/opt/trn_rl_repo", "/opt/pypackages"):
    if _p not in sys.path:
        sys.path.insert(0, _p)

import numpy as np

import concourse.bass as bass
import concourse.mybir as mybir
import concourse.tile as tile
from concourse import bacc
from concourse.bass_utils import run_bass_kernel_spmd

B = 2
S = 2048
D = 1024
H = 16
DH = 64
NCORES = 8
SB = 512           # q block (matmul moving dim)
KC = 128           # k chunk (contraction tile)
NSB = S // SB      # 4 q-blocks
NKC = S // KC      # 16 k-chunks
NDC = D // KC      # 8 contraction chunks for the projections

_USE_F32R = True

_compiled = None


def _build():
    f32 = mybir.dt.float32
    bf16 = mybir.dt.bfloat16
    fr = mybir.dt.float32r if _USE_F32R else f32
    nc = bacc.Bacc(None, target_bir_lowering=False)

    # host-blocked inputs: every [128, N] tile is contiguous in DRAM.
    # Matmul inputs are declared float32r: same 4-byte data, PE rounds
    # internally, and plain (non-casting) sync DMA is allowed.
    xt = nc.declare_dram_parameter("xt", [B, NSB, NDC, KC, SB], fr, isOutput=False)
    wqk = nc.declare_dram_parameter("wqk", [NDC, KC, 2 * KC], fr, isOutput=False)
    wv = nc.declare_dram_parameter("wv", [NDC, KC, 2 * KC], fr, isOutput=False)
    wout = nc.declare_dram_parameter("wout", [NDC, KC, D], fr, isOutput=False)
    bqk = nc.declare_dram_parameter("bqk", [2 * KC], f32, isOutput=False)
    bv = nc.declare_dram_parameter("bv", [2 * DH], f32, isOutput=False)
    bo = nc.declare_dram_parameter("bo", [D], f32, isOutput=False)
    vones = nc.declare_dram_parameter("vones", [KC, NKC], fr, isOutput=False)
    out_ext = nc.declare_dram_parameter("out", [SB, D], f32, isOutput=True)

    # AllToAll staging: block t -> core t gets my heads of batch t//4 for
    # s-quarter t%4.
    a2a_in = nc.dram_tensor("a2a_in", [NCORES, KC, SB], fr)
    a2a_out = nc.dram_tensor("a2a_out", [NCORES, KC, SB], fr)

    with tile.TileContext(nc) as tc:
        with (
            tc.tile_pool(name="qkv", bufs=1) as qkvp,
            tc.tile_pool(name="obuf", bufs=1) as op,
            tc.tile_pool(name="misc", bufs=1) as mp,
            tc.tile_pool(name="evict", bufs=1) as ep,
        ):
            # ---- small constants -----------------------------------------
            bqk_t = [mp.tile([KC, 1], f32, tag=f"bqk{m}", name=f"bqk{m}")
                     for m in range(2)]
            for m in range(2):
                nc.scalar.dma_start(
                    out=bqk_t[m][:],
                    in_=bqk[m * KC:(m + 1) * KC].rearrange("(p o) -> p o", o=1),
                )
            bv_row = mp.tile([1, 2 * DH], f32, tag="bv_row")
            nc.scalar.dma_start(out=bv_row[:], in_=bv.rearrange("(o f) -> o f", o=1))
            bv_bc = mp.tile([KC, 2 * DH], f32, tag="bv_bc")
            nc.gpsimd.partition_broadcast(out_ap=bv_bc[:], in_ap=bv_row[:])
            bo_row = mp.tile([1, D], f32, tag="bo_row")
            nc.scalar.dma_start(out=bo_row[:], in_=bo.rearrange("(o f) -> o f", o=1))
            bo_bc = mp.tile([KC, D], f32, tag="bo_bc")
            nc.gpsimd.partition_broadcast(out_ap=bo_bc[:], in_ap=bo_row[:])

            # ---- persistent activations ----------------------------------
            # pair p = batch p with heads (2c, 2c+1).
            # QQ[p]: rows 0:64 = Q^T of head 2c, rows 64:128 = head 2c+1
            # per-sblk tiles so attention can start before all of the
            # projection finishes (Tile deps are per-tile)
            QQ = [[qkvp.tile([KC, SB], fr, tag=f"QQ{p}_{s}", name=f"QQ{p}_{s}")
                   for s in range(NSB)] for p in range(2)]
            KK = [[qkvp.tile([KC, SB], fr, tag=f"KK{p}_{s}", name=f"KK{p}_{s}")
                   for s in range(NSB)] for p in range(2)]
            # V[2p+hh][s]: [128, 4*65]; chunk sc at cols sc*65..+64; col 64: 1.0
            NCS = SB // KC
            V = [[qkvp.tile([KC, NCS * (DH + 1)], fr, tag=f"V{v}_{s}",
                            name=f"V{v}_{s}")
                  for s in range(NSB)] for v in range(4)]
            vones_sb = mp.tile([KC, NKC], fr, tag="vones_sb")
            nc.scalar.dma_start(out=vones_sb[:], in_=vones[:])
            for v in range(4):
                for s in range(NSB):
                    vv = V[v][s][:].rearrange("p (k c) -> p k c", c=DH + 1)
                    nc.vector.tensor_copy(
                        vv[:, :, DH], vones_sb[:, s * NCS:(s + 1) * NCS])
            # O[p]: rows 0:64 = head 2c out^T (normalized), 64:128 = head 2c+1
            O = [op.tile([KC, S], fr, tag=f"O{p}", name=f"O{p}") for p in range(2)]

            # ---- phase 1: projections ------------------------------------
            with (
                tc.tile_pool(name="pjw", bufs=1) as wp,
                tc.tile_pool(name="xbuf", bufs=24) as xp,
                tc.tile_pool(name="psum_proj", bufs=1, space="PSUM") as pp,
            ):
                wqk_t = [wp.tile([KC, 2 * KC], fr, tag=f"wqk{k}", name=f"wqk{k}")
                         for k in range(NDC)]
                wv_t = [wp.tile([KC, 2 * KC], fr, tag=f"wv{k}", name=f"wv{k}")
                        for k in range(NDC)]
                for k in range(NDC):
                    nc.sync.dma_start(out=wqk_t[k][:], in_=wqk[k])

                for sblk in range(NSB):
                    for bb in range(B):
                        xs = []
                        for k in range(NDC):
                            xtl = xp.tile([KC, SB], fr, tag="xt")
                            eng = nc.sync if k % 2 == 0 else nc.gpsimd
                            eng.dma_start(out=xtl[:], in_=xt[bb, sblk, k])
                            xs.append(xtl)
                        # m-chunk 0 -> QQ[bb], 1 -> KK[bb]
                        for m in range(2):
                            ps = pp.tile([KC, SB], f32, tag="ps_qk", bufs=4)
                            for k in range(NDC):
                                nc.tensor.matmul(
                                    ps[:],
                                    wqk_t[k][:, m * KC:(m + 1) * KC],
                                    xs[k][:],
                                    start=(k == 0),
                                    stop=(k == NDC - 1),
                                )
                            dest = (QQ if m == 0 else KK)[bb][sblk]
                            nc.vector.tensor_scalar_add(
                                dest[:], ps[:], bqk_t[m][:],
                            )
                        if sblk == 0 and bb == 0:
                            # defer Wv loads so the first QK matmuls (which
                            # need only wqk + x) start as early as possible
                            for k in range(NDC):
                                nc.gpsimd.dma_start(out=wv_t[k][:], in_=wv[k])
                        # V natural: lhsT = x^T chunk; rhs = Wv (zero-padded
                        # to N=256 so f32r streams at full rate)
                        for sc in range(SB // KC):
                            ps = pp.tile([KC, 2 * KC], f32, tag="ps_v", bufs=4)
                            for k in range(NDC):
                                nc.tensor.matmul(
                                    ps[:],
                                    xs[k][:, sc * KC:(sc + 1) * KC],
                                    wv_t[k][:],
                                    start=(k == 0),
                                    stop=(k == NDC - 1),
                                )
                            for hh in range(2):
                                nc.vector.tensor_add(
                                    V[2 * bb + hh][sblk][:, sc * (DH + 1):
                                                         sc * (DH + 1) + DH],
                                    ps[:, hh * DH:(hh + 1) * DH],
                                    bv_bc[:, hh * DH:(hh + 1) * DH],
                                )

            # ---- phase 2: attention --------------------------------------
            with (
                tc.tile_pool(name="pbuf", bufs=1) as pb,
                tc.tile_pool(name="psum_att", bufs=1, space="PSUM") as pa,
            ):
                for qblk in range(NSB):
                    nkc = 4 * (qblk + 1)  # causal: k-chunks 0..nkc-1
                    P_all = []
                    for p in range(B):
                        # P[kc]: [128, 1024]; cols hh*512.. hold head hh
                        P = [
                            pb.tile([KC, 2 * SB], fr, tag=f"P{kc}",
                                    name=f"P{kc}_{p}_{qblk}",
                                    bufs=(2 if kc < 11 else 1))
                            for kc in range(nkc)
                        ]
                        P_all.append(P)
                        for kc in range(nkc):
                            d = kc - 4 * qblk
                            # causal: columns < 128*d are fully masked; skip
                            # them in the matmul/exp where the speed holds up
                            c0 = min(KC * max(d, 0), 2 * KC)
                            ps = pa.tile([KC, 2 * SB], f32, tag="ps_s", bufs=3)
                            for hh in range(2):  # row-tiled head pair
                                r0 = hh * DH
                                nc.tensor.matmul(
                                    ps[:, hh * SB + c0:(hh + 1) * SB],
                                    KK[p][kc // 4][r0:r0 + DH,
                                                   (kc % 4) * KC:
                                                   (kc % 4 + 1) * KC],
                                    QQ[p][qblk][r0:r0 + DH, c0:SB],
                                    start=True,
                                    stop=True,
                                )
                            ps3 = ps[:].rearrange("p (h f) -> p h f", h=2)
                            pd3 = P[kc][:].rearrange("p (h f) -> p h f", h=2)
                            e0 = KC * max(d, 0)
                            nc.scalar.activation(
                                pd3[:, :, e0:SB],
                                ps3[:, :, e0:SB],
                                mybir.ActivationFunctionType.Exp,
                                scale=1.0 / float(np.sqrt(DH)),
                            )
                            if d >= 0:  # diagonal chunk: zero where k > q
                                # only columns >= c0 are ever read by the AV
                                # matmul, so mask just that range
                                nc.gpsimd.affine_select(
                                    out=pd3[:, :, c0:SB],
                                    in_=pd3[:, :, c0:SB],
                                    pattern=[[0, 2], [1, SB - c0]],
                                    compare_op=mybir.AluOpType.is_ge,
                                    fill=0.0,
                                    base=c0 - KC * d,
                                    channel_multiplier=-1,
                                )
                    for p in range(B):
                        P = P_all[p]
                        pos = [pa.tile([DH + 1, SB], f32, tag=f"ps_av{hh}",
                                       bufs=1, name=f"po{hh}_{p}_{qblk}")
                               for hh in range(2)]
                        for kc in range(nkc):
                            d = kc - 4 * qblk
                            c0 = min(KC * max(d, 0), 2 * KC)
                            for hh in range(2):
                                nc.tensor.matmul(
                                    pos[hh][:, c0:SB],
                                    V[2 * p + hh][kc // 4][:,
                                        (kc % 4) * (DH + 1):
                                        (kc % 4 + 1) * (DH + 1)],
                                    P[kc][:, hh * SB + c0:(hh + 1) * SB],
                                    start=(kc == 0),
                                    stop=(kc == nkc - 1),
                                )
                        for hh in range(2):
                            po = pos[hh]
                            # free the psum bank immediately; normalize later
                            avst = ep.tile([DH + 1, SB], f32, tag="avst", bufs=4)
                            nc.vector.tensor_copy(avst[:], po[:])
                            den0 = ep.tile([1, SB], f32, tag="den0", bufs=1)
                            nc.vector.tensor_copy(den0[:], avst[DH:DH + 1, :])
                            rden = ep.tile([1, SB], f32, tag="rden", bufs=1)
                            rscr = ep.tile([1, SB], f32, tag="rscr", bufs=1)
                            nc.vector.reciprocal_approx_accurate(
                                rden[:], den0[:], rscr[:])
                            rden_bc = ep.tile([DH, SB], f32, tag="rden_bc", bufs=2)
                            nc.gpsimd.partition_broadcast(
                                out_ap=rden_bc[:], in_ap=rden[:]
                            )
                            r0 = hh * DH
                            nc.vector.tensor_mul(
                                O[p][r0:r0 + DH, qblk * SB:(qblk + 1) * SB],
                                avst[0:DH, :],
                                rden_bc[:],
                            )
                        # stage this (batch, quarter) block for the AllToAll
                        nc.sync.dma_start(
                            out=a2a_in[4 * p + qblk],
                            in_=O[p][:, qblk * SB:(qblk + 1) * SB],
                        )

            # ---- phase 3: head exchange + output projection --------------
            nc.gpsimd.collective_compute(
                "AllToAll",
                mybir.AluOpType.bypass,
                replica_groups=[[0, 1, 2, 3, 4, 5, 6, 7]],
                ins=[a2a_in[:]],
                outs=[a2a_out[:]],
            )
            with (
                tc.tile_pool(name="wout_pool", bufs=1) as wop,
                tc.tile_pool(name="recv", bufs=1) as rp,
                tc.tile_pool(name="psum_out", bufs=1, space="PSUM") as pu,
            ):
                wout_t = [wop.tile([KC, D], fr, tag=f"wo{k}", name=f"wo{k}")
                          for k in range(NDC)]
                for k in range(NDC):
                    nc.sync.dma_start(out=wout_t[k][:], in_=wout[k])
                # a2a_out block i = heads (2i, 2i+1) of my batch for my
                # quarter -> flat [1024, 512] = attnout^T in global head order
                recv = [rp.tile([KC, SB], fr, tag=f"rc{k}", name=f"rc{k}")
                        for k in range(NDC)]
                for k in range(NDC):
                    eng = nc.sync if k % 2 == 0 else nc.gpsimd
                    eng.dma_start(out=recv[k][:], in_=a2a_out[k])
                for sc in range(SB // KC):
                    for nb in range(D // SB):
                        ps = pu.tile([KC, SB], f32, tag="ps_o", bufs=4)
                        for k in range(NDC):
                            nc.tensor.matmul(
                                ps[:],
                                recv[k][:, sc * KC:(sc + 1) * KC],
                                wout_t[k][:, nb * SB:(nb + 1) * SB],
                                start=(k == 0),
                                stop=(k == NDC - 1),
                            )
                        ot = ep.tile([KC, SB], f32, tag="osb", bufs=4)
                        nc.vector.tensor_add(
                            ot[:], ps[:], bo_bc[:, nb * SB:(nb + 1) * SB]
                        )
                        nc.sync.dma_start(
                            out=out_ext[sc * KC:(sc + 1) * KC,
                                        nb * SB:(nb + 1) * SB],
                            in_=ot[:],
                        )

    nc.compile()
    return nc


def _get_program():
    global _compiled
    if _compiled is None:
        _compiled = _build()
    return _compiled


def _shard_inputs(x, Wqkv, bqkv, Wout, bout):
    """Build the 8 per-core input maps (all host-side numpy)."""
    x = np.ascontiguousarray(x, dtype=np.float32)
    Wqkv = np.asarray(Wqkv, dtype=np.float32)
    bqkv = np.asarray(bqkv, dtype=np.float32)
    Wout = np.asarray(Wout, dtype=np.float32)
    bout = np.ascontiguousarray(np.asarray(bout, dtype=np.float32))

    Wq = Wqkv[:, 0 * D:1 * D]
    Wk = Wqkv[:, 1 * D:2 * D]
    Wv_full = Wqkv[:, 2 * D:3 * D]
    bq = bqkv[0 * D:1 * D]
    bk = bqkv[1 * D:2 * D]
    bv_full = bqkv[2 * D:3 * D]

    # shared across all cores
    xt = np.ascontiguousarray(
        x.transpose(0, 2, 1)                      # [B, D, S]
         .reshape(B, D, NSB, SB).transpose(0, 2, 1, 3)
         .reshape(B, NSB, NDC, KC, SB)
    )
    wout_b = np.ascontiguousarray(Wout.reshape(NDC, KC, D))
    vones = np.ones((KC, NKC), dtype=np.float32)

    in_maps = []
    for c in range(NCORES):
        ha, hb = 2 * c, 2 * c + 1
        wqk_c = np.ascontiguousarray(np.concatenate(
            [Wq[:, ha * DH:(ha + 1) * DH], Wq[:, hb * DH:(hb + 1) * DH],
             Wk[:, ha * DH:(ha + 1) * DH], Wk[:, hb * DH:(hb + 1) * DH]],
            axis=1).reshape(NDC, KC, 2 * KC))
        bqk_c = np.ascontiguousarray(np.concatenate(
            [bq[ha * DH:(ha + 1) * DH], bq[hb * DH:(hb + 1) * DH],
             bk[ha * DH:(ha + 1) * DH], bk[hb * DH:(hb + 1) * DH]]))
        # Wv zero-padded to 256 columns so the V matmul moving dim is 256
        wv_c = np.zeros((D, 2 * KC), dtype=np.float32)
        wv_c[:, 0:DH] = Wv_full[:, ha * DH:(ha + 1) * DH]
        wv_c[:, DH:2 * DH] = Wv_full[:, hb * DH:(hb + 1) * DH]
        wv_c = np.ascontiguousarray(wv_c.reshape(NDC, KC, 2 * KC))
        bv_c = np.ascontiguousarray(np.concatenate(
            [bv_full[ha * DH:(ha + 1) * DH], bv_full[hb * DH:(hb + 1) * DH]]))
        in_maps.append({
            "xt": xt, "wqk": wqk_c, "wv": wv_c, "wout": wout_b,
            "bqk": bqk_c, "bv": bv_c, "bo": bout, "vones": vones,
        })
    return in_maps


def run(inputs, trace=False, trace_kwargs=None):
    nc = _get_program()
    in_maps = _shard_inputs(**inputs)
    res = run_bass_kernel_spmd(
        nc, in_maps, list(range(NCORES)), trace=trace,
        **(trace_kwargs or {}),
    )
    out = np.empty((B, S, D), dtype=np.float32)
    for c in range(NCORES):
        b = c // 4
        r0 = SB * (c % 4)
        out[b, r0:r0 + SB, :] = res.results[c]["out"]
    return out, res


def kernel(**inputs):
    out, _ = run(inputs)
    return out



# revision 12
# speedup vs baseline: 1.2141x; 1.2141x over previous
"""Causal multi-head attention (B=2, S=2048, D=1024, H=16) on 8 trn2 cores.

Sharding v2: core c handles batch b = c//4 and heads {4r..4r+3} (r = c%4),
i.e. cores 0-3 cover batch 0, cores 4-7 batch 1.  Per core:

  - project the host-pretransposed x_b^T [D, S] (OWN batch only) through the
    core's Wqkv column slice into Q^T/K^T head-pair tiles (fp16) and V in
    natural layout with a fused ones-column (so the AV matmul also emits the
    softmax denominators),
  - causal attention per (head-pair, qblock) in transposed layout, fp16
    operands with fp32 PSUM accumulation: scores^T = K Q^T (row-tiled head
    pairs), exp on ScalarE, diagonal masks on GpSimd, A^T V on PE,
  - after each 512-query block, a 4-wide AllToAll (replica groups
    [0-3],[4-7]) redistributes that quarter's head outputs so core (b,r)
    receives ALL 16 heads for queries 512*q + 128*r .. +128; these four
    collectives overlap with the remaining attention compute,
  - the output projection through the full Wout (fp16) runs per received
    128-query chunk, pipelined behind the collectives.

Host assembles the 8 cores x 4 chunks of [128, 1024] into (2, 2048, 1024).

Projection matmuls run in float32r (TF32-like, ~1e-3 rel err); everything
downstream of the projections is fp16 (≥10-bit mantissa, same PE throughput,
half the SBUF/DMA/collective bytes).
"""

import sys

for _p in ("/opt/trn_rl_repo", "/opt/pypackages"):
    if _p not in sys.path:
        sys.path.insert(0, _p)

import numpy as np

import concourse.bass as bass
import concourse.mybir as mybir
import concourse.tile as tile
from concourse import bacc
from concourse.bass_utils import run_bass_kernel_spmd

B = 2
S = 2048
D = 1024
H = 16
DH = 64
NCORES = 8
SB = 512           # q block (matmul moving dim)
KC = 128           # k chunk (contraction tile)
NSB = S // SB      # 4 q-blocks
NKC = S // KC      # 16 k-chunks
NDC = D // KC      # 8 contraction chunks for the projections

_compiled = None


def _build():
    f32 = mybir.dt.float32
    f16 = mybir.dt.float16
    fr = mybir.dt.float32r
    nc = bacc.Bacc(None, target_bir_lowering=False)

    # host-blocked inputs (own batch / own 4 heads only)
    xt = nc.declare_dram_parameter("xt", [NSB, NDC, KC, SB], fr, isOutput=False)
    wqk = nc.declare_dram_parameter("wqk", [NDC, KC, 4 * KC], fr, isOutput=False)
    wv = nc.declare_dram_parameter("wv", [NDC, KC, 2 * KC], fr, isOutput=False)
    wout = nc.declare_dram_parameter("wout", [2, NDC, KC, SB], f16, isOutput=False)
    bqk = nc.declare_dram_parameter("bqk", [KC, 4], f32, isOutput=False)
    bv = nc.declare_dram_parameter("bv", [1, 2 * KC], f32, isOutput=False)
    bo = nc.declare_dram_parameter("bo", [1, D], f32, isOutput=False)
    vones = nc.declare_dram_parameter("vones", [KC, 16], f16, isOutput=False)
    out_ext = nc.declare_dram_parameter("out", [NSB, KC, D], f32, isOutput=True)

    # per-quarter AllToAll staging: a2a_in_q[t, hp] = this core's head-pair hp
    # output (transposed, [128 head dims, 128 queries]) for query sub-chunk
    # 512*q + 128*(t%4).  The exchange is logically within each batch group
    # of 4 cores, but the collective stack only supports 8-wide AllToAll
    # (mesh), so both group halves carry the same data and each receiver
    # dynamically slices its own group's 4 sender blocks.
    a2a_in = [nc.dram_tensor(f"a2a_in{q}", [NCORES, 2, KC, KC], f16)
              for q in range(NSB)]
    a2a_out = [nc.dram_tensor(f"a2a_out{q}", [NCORES, 2, KC, KC], f16)
               for q in range(NSB)]
    groups = [[0, 1, 2, 3, 4, 5, 6, 7]]

    with tile.TileContext(nc) as tc:
        with (
            tc.tile_pool(name="misc", bufs=1) as mp,
            tc.tile_pool(name="weights", bufs=1) as wp,
            tc.tile_pool(name="xbuf", bufs=1) as xp,
            tc.tile_pool(name="qkv", bufs=1) as qkvp,
            tc.tile_pool(name="pbuf", bufs=1) as pb,
            tc.tile_pool(name="obuf", bufs=1) as op,
            tc.tile_pool(name="evict", bufs=1) as ep,
            tc.tile_pool(name="psum", bufs=1, space="PSUM") as pp,
        ):
            # ---- PE warmup: keep the HAM clock ramping while DMAs land ----
            wdum = mp.tile([KC, KC], f16, tag="wdum")
            nc.vector.memset(wdum[:], 0.0)
            for i in range(48):
                psd = pp.tile([KC, SB], f32, tag="accum", bufs=2)
                nc.tensor.matmul(psd[:, 0:KC], wdum[:], wdum[:],
                                 start=True, stop=True)

            # ---- big loads first: unblock the first projection ASAP -------
            # wqk_t cols: k*512 + m*128, m in {Q01, Q23, K01, K23}
            wqk_t = wp.tile([KC, NDC * 4 * KC], fr, tag="wqk")
            nc.gpsimd.dma_start(
                out=wqk_t[:].rearrange("p (k c) -> p k c", k=NDC),
                in_=wqk[:].rearrange("k p c -> p k c"),
            )
            xts = []
            for s in range(NSB):
                xts.append(xp.tile([KC, NDC * SB], fr, tag=f"xt{s}",
                                   name=f"xt{s}"))
            nc.gpsimd.dma_start(
                out=xts[0][:].rearrange("p (k c) -> p k c", k=NDC),
                in_=xt[0].rearrange("k p c -> p k c"),
            )
            wv_t = wp.tile([KC, NDC * 2 * KC], fr, tag="wv")
            nc.gpsimd.dma_start(
                out=wv_t[:].rearrange("p (k c) -> p k c", k=NDC),
                in_=wv[:].rearrange("k p c -> p k c"),
            )
            nc.gpsimd.dma_start(
                out=xts[1][:].rearrange("p (k c) -> p k c", k=NDC),
                in_=xt[1].rearrange("k p c -> p k c"),
            )

            # ---- small constants -----------------------------------------
            bqk_t = mp.tile([KC, 4], f32, tag="bqk")
            nc.sync.dma_start(out=bqk_t[:], in_=bqk[:])
            bv_row = mp.tile([1, 2 * KC], f32, tag="bv_row")
            nc.sync.dma_start(out=bv_row[:], in_=bv[:])
            bv_bc = mp.tile([KC, 2 * KC], f32, tag="bv_bc")
            nc.gpsimd.partition_broadcast(out_ap=bv_bc[:], in_ap=bv_row[:])
            bo_row = mp.tile([1, D], f32, tag="bo_row")
            nc.sync.dma_start(out=bo_row[:], in_=bo[:])
            bo_bc = mp.tile([KC, D], f32, tag="bo_bc")
            nc.gpsimd.partition_broadcast(out_ap=bo_bc[:], in_ap=bo_row[:])
            vones_sb = mp.tile([KC, 16], f16, tag="vones_sb")
            nc.sync.dma_start(out=vones_sb[:], in_=vones[:])

            # ---- persistent activations ----------------------------------
            # QQ[hp][s]: rows 0:64 = Q^T head 4r+2hp, 64:128 = head 4r+2hp+1
            QQ = [[qkvp.tile([KC, SB], f16, tag=f"QQ{hp}_{s}",
                             name=f"QQ{hp}_{s}") for s in range(NSB)]
                  for hp in range(2)]
            KK = [[qkvp.tile([KC, SB], f16, tag=f"KK{hp}_{s}",
                             name=f"KK{hp}_{s}") for s in range(NSB)]
                  for hp in range(2)]
            # V_all[s]: [128, 4 heads * 4 sc * 65]; head v block at v*260,
            # chunk sc at v*260 + sc*65, col 64 of each chunk = 1.0
            NCS = SB // KC
            V_all = [qkvp.tile([KC, 4 * NCS * (DH + 1)], f16, tag=f"V{s}",
                               name=f"V{s}") for s in range(NSB)]
            for s in range(NSB):
                vv = V_all[s][:].rearrange("p (v c) -> p v c", c=DH + 1)
                nc.vector.tensor_copy(vv[:, :, DH], vones_sb[:])
            # O[hp]: rows 0:64 = head 4r+2hp out^T (normalized), 64:128 =
            # head 4r+2hp+1
            O = [op.tile([KC, S], f16, tag=f"O{hp}", name=f"O{hp}")
                 for hp in range(2)]
            # wout (fp16, 2MB) loaded whole; needed from first out-proj on
            wout_t = wp.tile([KC, 2 * NDC * SB], f16, tag="wout")

            def proj(s):
                """QKV projection for seq block s (f32r)."""
                xs = xts[s]
                for m in range(4):
                    ps = pp.tile([KC, SB], f32, tag="accum", bufs=2,
                                 name=f"psqk{m}_{s}")
                    for k in range(NDC):
                        nc.tensor.matmul(
                            ps[:],
                            wqk_t[:, k * 4 * KC + m * KC:
                                  k * 4 * KC + (m + 1) * KC],
                            xs[:, k * SB:(k + 1) * SB],
                            start=(k == 0),
                            stop=(k == NDC - 1),
                        )
                        if s == 0 and m == 0 and k == 3:
                            # early wout kick: overlaps with projections
                            nc.gpsimd.dma_start(
                                out=wout_t[:].rearrange(
                                    "p (k c) -> p k c", k=2 * NDC),
                                in_=wout[:].rearrange(
                                    "n k p c -> p (n k) c"),
                            )
                    dest = (QQ if m < 2 else KK)[m % 2][s]
                    nc.vector.tensor_scalar_add(
                        dest[:], ps[:], bqk_t[:, m:m + 1])
                for sc in range(NCS):
                    pv = pp.tile([KC, 2 * KC], f32, tag="accum", bufs=2,
                                 name=f"psv{sc}_{s}")
                    for k in range(NDC):
                        nc.tensor.matmul(
                            pv[:],
                            xs[:, k * SB + sc * KC:k * SB + (sc + 1) * KC],
                            wv_t[:, k * 2 * KC:(k + 1) * 2 * KC],
                            start=(k == 0),
                            stop=(k == NDC - 1),
                        )
                    vv = V_all[s][:].rearrange(
                        "p (v k c) -> p v k c", v=4, k=NCS)
                    nc.vector.tensor_add(
                        vv[:, :, sc, 0:DH],
                        pv[:].rearrange("p (v c) -> p v c", c=DH),
                        bv_bc[:].rearrange("p (v c) -> p v c", c=DH),
                    )

            def attn(q):
                """Attention for query block q, both head pairs."""
                nkc = 4 * (q + 1)
                for hp in range(2):
                    P = []
                    for kc in range(nkc):
                        d = kc - 4 * q
                        c0 = KC * max(d, 0)
                        ps = pp.tile([KC, 2 * SB], f32, tag="pss", bufs=2,
                                     name=f"pss{hp}_{q}_{kc}")
                        for hh in range(2):
                            r0 = hh * DH
                            nc.tensor.matmul(
                                ps[:, hh * SB + c0:(hh + 1) * SB],
                                KK[hp][kc // 4][r0:r0 + DH,
                                                (kc % 4) * KC:
                                                (kc % 4 + 1) * KC],
                                QQ[hp][q][r0:r0 + DH, c0:SB],
                                start=True,
                                stop=True,
                            )
                        pt = pb.tile([KC, 2 * SB], f16, tag="P", bufs=8,
                                     name=f"P{hp}_{q}_{kc}")
                        P.append(pt)
                        ps3 = ps[:].rearrange("p (h f) -> p h f", h=2)
                        pd3 = pt[:].rearrange("p (h f) -> p h f", h=2)
                        nc.scalar.activation(
                            pd3[:, :, c0:SB],
                            ps3[:, :, c0:SB],
                            mybir.ActivationFunctionType.Exp,
                            scale=1.0 / float(np.sqrt(DH)),
                        )
                        if d >= 0:  # diagonal chunk: zero where k > q
                            nc.gpsimd.affine_select(
                                out=pd3[:, :, c0:SB],
                                in_=pd3[:, :, c0:SB],
                                pattern=[[0, 2], [1, SB - c0]],
                                compare_op=mybir.AluOpType.is_ge,
                                fill=0.0,
                                base=0,
                                channel_multiplier=-1,
                            )
                    pos = [pp.tile([DH + 1, SB], f32, tag="pos", bufs=2,
                                   name=f"pos{hh}_{hp}_{q}")
                           for hh in range(2)]
                    for kc in range(nkc):
                        d = kc - 4 * q
                        c0 = KC * max(d, 0)
                        for hh in range(2):
                            v = 2 * hp + hh
                            nc.tensor.matmul(
                                pos[hh][:, c0:SB],
                                V_all[kc // 4][:, v * NCS * (DH + 1) +
                                               (kc % 4) * (DH + 1):
                                               v * NCS * (DH + 1) +
                                               (kc % 4 + 1) * (DH + 1)],
                                P[kc][:, hh * SB + c0:(hh + 1) * SB],
                                start=(kc == 0),
                                stop=(kc == nkc - 1),
                            )
                    # normalize: O = pos[0:64] * (1 / pos[64]) per head
                    for hh in range(2):
                        den0 = ep.tile([1, SB], f32, tag=f"den{hh}", bufs=2,
                                       name=f"den{hh}_{hp}_{q}")
                        nc.vector.tensor_copy(den0[:], pos[hh][DH:DH + 1, :])
                        rden = ep.tile([1, SB], f32, tag=f"rden{hh}", bufs=2)
                        rscr = ep.tile([1, SB], f32, tag=f"rscr{hh}", bufs=2)
                        nc.vector.reciprocal_approx_accurate(
                            rden[:], den0[:], rscr[:])
                        rbc = ep.tile([DH, SB], f32, tag=f"rbc{hh}", bufs=2,
                                      name=f"rbc{hh}_{hp}_{q}")
                        nc.gpsimd.partition_broadcast(
                            out_ap=rbc[:],
                            in_ap=rden[:],
                        )
                        r0 = hh * DH
                        nc.vector.tensor_mul(
                            O[hp][r0:r0 + DH, q * SB:(q + 1) * SB],
                            pos[hh][0:DH, :],
                            rbc[:],
                        )
                    # stage this head-pair's quarter for the AllToAll (both
                    # group halves get a copy)
                    for g in range(2):
                        nc.gpsimd.dma_start(
                            out=a2a_in[q][4 * g:4 * (g + 1), hp].rearrange(
                                "t p c -> p t c"),
                            in_=O[hp][:, q * SB:(q + 1) * SB].rearrange(
                                "p (t c) -> p t c", t=4),
                        )

            def collective(q):
                nc.gpsimd.collective_compute(
                    "AllToAll",
                    mybir.AluOpType.bypass,
                    replica_groups=groups,
                    ins=[a2a_in[q][:]],
                    outs=[a2a_out[q][:]],
                )

            # my group's sender blocks start at slot 4*(rank//4)
            pid = nc.sync.partition_id()
            soff = (pid // 4) * 4

            def outproj(q):
                """Project the received 128-query chunk of quarter q."""
                recv = ep.tile([KC, NDC * KC], f16, tag="recv", bufs=2,
                               name=f"recv{q}")
                nc.sync.dma_start(
                    out=recv[:].rearrange("p (k c) -> p k c", k=NDC),
                    in_=a2a_out[q][bass.ds(soff, 4)].rearrange(
                        "s h p c -> p (s h) c"),
                )
                for nb in range(2):
                    po = pp.tile([KC, SB], f32, tag="accum", bufs=2,
                                 name=f"po{nb}_{q}")
                    for k in range(NDC):
                        nc.tensor.matmul(
                            po[:],
                            recv[:, k * KC:(k + 1) * KC],
                            wout_t[:, (nb * NDC + k) * SB:
                                   (nb * NDC + k + 1) * SB],
                            start=(k == 0),
                            stop=(k == NDC - 1),
                        )
                    ot = ep.tile([KC, SB], f32, tag="osb", bufs=2,
                                 name=f"ot{nb}_{q}")
                    nc.vector.tensor_add(
                        ot[:], po[:], bo_bc[:, nb * SB:(nb + 1) * SB])
                    nc.sync.dma_start(
                        out=out_ext[q, :, nb * SB:(nb + 1) * SB],
                        in_=ot[:],
                    )

            # ---- pipeline -------------------------------------------------
            proj(0)
            nc.gpsimd.dma_start(
                out=xts[2][:].rearrange("p (k c) -> p k c", k=NDC),
                in_=xt[2].rearrange("k p c -> p k c"),
            )
            proj(1)
            nc.gpsimd.dma_start(
                out=xts[3][:].rearrange("p (k c) -> p k c", k=NDC),
                in_=xt[3].rearrange("k p c -> p k c"),
            )
            attn(0)
            collective(0)
            proj(2)
            attn(1)
            collective(1)
            outproj(0)
            proj(3)
            attn(2)
            collective(2)
            outproj(1)
            attn(3)
            collective(3)
            outproj(2)
            outproj(3)

    nc.compile()
    return nc


def _get_program():
    global _compiled
    if _compiled is None:
        _compiled = _build()
    return _compiled


def _shard_inputs(x, Wqkv, bqkv, Wout, bout):
    """Build the 8 per-core input maps (all host-side numpy)."""
    x = np.ascontiguousarray(x, dtype=np.float32)
    Wqkv = np.asarray(Wqkv, dtype=np.float32)
    bqkv = np.asarray(bqkv, dtype=np.float32)
    Wout = np.asarray(Wout, dtype=np.float32)
    bout = np.ascontiguousarray(np.asarray(bout, dtype=np.float32))

    Wq = Wqkv[:, 0 * D:1 * D]
    Wk = Wqkv[:, 1 * D:2 * D]
    Wv_full = Wqkv[:, 2 * D:3 * D]
    bq = bqkv[0 * D:1 * D]
    bk = bqkv[1 * D:2 * D]
    bv_full = bqkv[2 * D:3 * D]

    # per batch: [NSB, NDC, KC, SB] blocked transpose of x
    xts = []
    for b in range(B):
        xts.append(np.ascontiguousarray(
            x[b].T                                   # [D, S]
            .reshape(NDC, KC, NSB, SB).transpose(2, 0, 1, 3)
        ))
    wout_b = np.ascontiguousarray(
        Wout.reshape(NDC, KC, 2, SB).transpose(2, 0, 1, 3)
        .astype(np.float16))
    bo_row = np.ascontiguousarray(bout.reshape(1, D))
    vones = np.ones((KC, 16), dtype=np.float16)

    in_maps = []
    for c in range(NCORES):
        b, r = c // 4, c % 4
        h0 = 4 * r
        cols = lambda W, i: W[:, (h0 + i) * DH:(h0 + i + 2) * DH]
        wqk_c = np.ascontiguousarray(np.concatenate(
            [cols(Wq, 0), cols(Wq, 2), cols(Wk, 0), cols(Wk, 2)],
            axis=1).reshape(NDC, KC, 4 * KC))
        bqk_c = np.ascontiguousarray(np.stack(
            [bq[(h0) * DH:(h0 + 2) * DH], bq[(h0 + 2) * DH:(h0 + 4) * DH],
             bk[(h0) * DH:(h0 + 2) * DH], bk[(h0 + 2) * DH:(h0 + 4) * DH]],
            axis=1))
        wv_c = np.ascontiguousarray(
            Wv_full[:, h0 * DH:(h0 + 4) * DH].reshape(NDC, KC, 2 * KC))
        bv_c = np.ascontiguousarray(
            bv_full[h0 * DH:(h0 + 4) * DH].reshape(1, 2 * KC))
        in_maps.append({
            "xt": xts[b], "wqk": wqk_c, "wv": wv_c, "wout": wout_b,
            "bqk": bqk_c, "bv": bv_c, "bo": bo_row, "vones": vones,
        })
    return in_maps


def run(inputs, trace=False, trace_kwargs=None):
    nc = _get_program()
    in_maps = _shard_inputs(**inputs)
    res = run_bass_kernel_spmd(
        nc, in_maps, list(range(NCORES)), trace=trace,
        **(trace_kwargs or {}),
    )
    out = np.empty((B, S, D), dtype=np.float32)
    for c in range(NCORES):
        b, r = c // 4, c % 4
        o = res.results[c]["out"]          # [NSB, KC, D]
        for q in range(NSB):
            out[b, SB * q + KC * r:SB * q + KC * (r + 1), :] = o[q]
    return out, res


def kernel(**inputs):
    out, _ = run(inputs)
    return out


# revision 16
# speedup vs baseline: 1.3049x; 1.0748x over previous
"""Causal multi-head attention (B=2, S=2048, D=1024, H=16) on 8 trn2 cores.

Sharding v2: core c handles batch b = c//4 and heads {4r..4r+3} (r = c%4),
i.e. cores 0-3 cover batch 0, cores 4-7 batch 1.  Per core:

  - project the host-pretransposed x_b^T [D, S] (OWN batch only) through the
    core's Wqkv column slice into Q^T/K^T head-pair tiles (fp16) and V in
    natural layout with a fused ones-column (so the AV matmul also emits the
    softmax denominators),
  - causal attention per (head-pair, qblock) in transposed layout, fp16
    operands with fp32 PSUM accumulation: scores^T = K Q^T (row-tiled head
    pairs), exp on ScalarE, diagonal masks on GpSimd, A^T V on PE,
  - after each 512-query block, a 4-wide AllToAll (replica groups
    [0-3],[4-7]) redistributes that quarter's head outputs so core (b,r)
    receives ALL 16 heads for queries 512*q + 128*r .. +128; these four
    collectives overlap with the remaining attention compute,
  - the output projection through the full Wout (fp16) runs per received
    128-query chunk, pipelined behind the collectives.

Host assembles the 8 cores x 4 chunks of [128, 1024] into (2, 2048, 1024).

Projection matmuls run in float32r (TF32-like, ~1e-3 rel err); everything
downstream of the projections is fp16 (≥10-bit mantissa, same PE throughput,
half the SBUF/DMA/collective bytes).
"""

import sys

for _p in ("/opt/trn_rl_repo", "/opt/pypackages"):
    if _p not in sys.path:
        sys.path.insert(0, _p)

import numpy as np

import concourse.bass as bass
import concourse.mybir as mybir
import concourse.tile as tile
from concourse import bacc
from concourse.bass_utils import run_bass_kernel_spmd

B = 2
S = 2048
D = 1024
H = 16
DH = 64
NCORES = 8
SB = 512           # q block (matmul moving dim)
KC = 128           # k chunk (contraction tile)
NSB = S // SB      # 4 q-blocks
NKC = S // KC      # 16 k-chunks
NDC = D // KC      # 8 contraction chunks for the projections

_compiled = None


def _build():
    f32 = mybir.dt.float32
    f16 = mybir.dt.float16
    fr = mybir.dt.float32r
    nc = bacc.Bacc(None, target_bir_lowering=False)

    # host-blocked inputs (own batch / own 4 heads only)
    xt = nc.declare_dram_parameter("xt", [NSB, NDC, KC, SB], fr, isOutput=False)
    wqk = nc.declare_dram_parameter("wqk", [NDC, KC, 4 * KC], fr, isOutput=False)
    wv = nc.declare_dram_parameter("wv", [NDC, KC, 2 * KC], fr, isOutput=False)
    wout = nc.declare_dram_parameter("wout", [2, NDC, KC, SB], f16, isOutput=False)
    bqk = nc.declare_dram_parameter("bqk", [KC, 4], f32, isOutput=False)
    bv = nc.declare_dram_parameter("bv", [1, 2 * KC], f32, isOutput=False)
    bo = nc.declare_dram_parameter("bo", [1, D], f32, isOutput=False)
    vones = nc.declare_dram_parameter("vones", [KC, 16], f16, isOutput=False)
    out_ext = nc.declare_dram_parameter("out", [NSB, KC, D], f32, isOutput=True)

    # per-quarter AllToAll staging: a2a_in_q[t, hp] = this core's head-pair hp
    # output (transposed, [128 head dims, 128 queries]) for query sub-chunk
    # 512*q + 128*(t%4).  The exchange is logically within each batch group
    # of 4 cores, but the collective stack only supports 8-wide AllToAll
    # (mesh), so both group halves carry the same data and each receiver
    # dynamically slices its own group's 4 sender blocks.
    a2a_in = [nc.dram_tensor(f"a2a_in{q}", [NCORES, 2, KC, KC], f16)
              for q in range(NSB)]
    a2a_out = [nc.dram_tensor(f"a2a_out{q}", [NCORES, 2, KC, KC], f16)
               for q in range(NSB)]
    groups = [[0, 1, 2, 3, 4, 5, 6, 7]]
    # tiny warm-up collective: absorbs the ~15us first-collective cold cost
    # during the initial DMA wait
    cc_warm_in = nc.dram_tensor("cc_warm_in", [NCORES, 16], f16)
    cc_warm_out = nc.dram_tensor("cc_warm_out", [NCORES, 16], f16)

    with tile.TileContext(nc) as tc:
        with (
            tc.tile_pool(name="misc", bufs=1) as mp,
            tc.tile_pool(name="weights", bufs=1) as wp,
            tc.tile_pool(name="xbuf", bufs=1) as xp,
            tc.tile_pool(name="qkv", bufs=1) as qkvp,
            tc.tile_pool(name="pbuf", bufs=1) as pb,
            tc.tile_pool(name="obuf", bufs=1) as op,
            tc.tile_pool(name="evict", bufs=1) as ep,
            tc.tile_pool(name="psum", bufs=1, space="PSUM") as pp,
        ):
            # ---- CC + PE warmup while the initial DMAs land ---------------
            nc.gpsimd.collective_compute(
                "AllToAll",
                mybir.AluOpType.bypass,
                replica_groups=groups,
                ins=[cc_warm_in[:]],
                outs=[cc_warm_out[:]],
            )
            wdum = mp.tile([KC, KC], f16, tag="wdum")
            nc.vector.memset(wdum[:], 0.0)
            for i in range(96):
                psd = pp.tile([KC, SB], f32, tag="accum", bufs=2)
                nc.tensor.matmul(psd[:, 0:KC], wdum[:], wdum[:],
                                 start=True, stop=True)

            # ---- big loads first: unblock the first projection ASAP -------
            # wqk_t cols: k*512 + m*128, m in {Q01, Q23, K01, K23}; first
            # halves (k 0:4) land first so proj(0) can begin sooner
            wqk_t = wp.tile([KC, NDC * 4 * KC], fr, tag="wqk")
            xts = []
            for s in range(NSB):
                xts.append(xp.tile([KC, NDC * SB], fr, tag=f"xt{s}",
                                   name=f"xt{s}"))
            HD = NDC // 2
            for h in range(2):
                nc.gpsimd.dma_start(
                    out=wqk_t[:, h * HD * 4 * KC:(h + 1) * HD * 4 * KC]
                    .rearrange("p (k c) -> p k c", k=HD),
                    in_=wqk[h * HD:(h + 1) * HD].rearrange("k p c -> p k c"),
                )
                nc.gpsimd.dma_start(
                    out=xts[0][:, h * HD * SB:(h + 1) * HD * SB]
                    .rearrange("p (k c) -> p k c", k=HD),
                    in_=xt[0, h * HD:(h + 1) * HD].rearrange("k p c -> p k c"),
                )
            wv_t = wp.tile([KC, NDC * 2 * KC], fr, tag="wv")
            nc.gpsimd.dma_start(
                out=wv_t[:].rearrange("p (k c) -> p k c", k=NDC),
                in_=wv[:].rearrange("k p c -> p k c"),
            )
            nc.gpsimd.dma_start(
                out=xts[1][:].rearrange("p (k c) -> p k c", k=NDC),
                in_=xt[1].rearrange("k p c -> p k c"),
            )

            # ---- small constants -----------------------------------------
            bqk_t = mp.tile([KC, 4], f32, tag="bqk")
            nc.sync.dma_start(out=bqk_t[:], in_=bqk[:])
            bv_row = mp.tile([1, 2 * KC], f32, tag="bv_row")
            nc.sync.dma_start(out=bv_row[:], in_=bv[:])
            bv_bc = mp.tile([KC, 2 * KC], f32, tag="bv_bc")
            nc.gpsimd.partition_broadcast(out_ap=bv_bc[:], in_ap=bv_row[:])
            bo_row = mp.tile([1, D], f32, tag="bo_row")
            nc.sync.dma_start(out=bo_row[:], in_=bo[:])
            bo_bc = mp.tile([KC, D], f32, tag="bo_bc")
            nc.gpsimd.partition_broadcast(out_ap=bo_bc[:], in_ap=bo_row[:])
            vones_sb = mp.tile([KC, 16], f16, tag="vones_sb")
            nc.sync.dma_start(out=vones_sb[:], in_=vones[:])

            # ---- persistent activations ----------------------------------
            # QQ[hp][s]: rows 0:64 = Q^T head 4r+2hp, 64:128 = head 4r+2hp+1
            QQ = [[qkvp.tile([KC, SB], f16, tag=f"QQ{hp}_{s}",
                             name=f"QQ{hp}_{s}") for s in range(NSB)]
                  for hp in range(2)]
            KK = [[qkvp.tile([KC, SB], f16, tag=f"KK{hp}_{s}",
                             name=f"KK{hp}_{s}") for s in range(NSB)]
                  for hp in range(2)]
            # V_all[s]: [128, 4 heads * 4 sc * 65]; head v block at v*260,
            # chunk sc at v*260 + sc*65, col 64 of each chunk = 1.0
            NCS = SB // KC
            V_all = [qkvp.tile([KC, 4 * NCS * (DH + 1)], f16, tag=f"V{s}",
                               name=f"V{s}") for s in range(NSB)]
            for s in range(NSB):
                vv = V_all[s][:].rearrange("p (v c) -> p v c", c=DH + 1)
                nc.vector.tensor_copy(vv[:, :, DH], vones_sb[:])
            # O[hp]: rows 0:64 = head 4r+2hp out^T (normalized), 64:128 =
            # head 4r+2hp+1
            O = [op.tile([KC, S], f16, tag=f"O{hp}", name=f"O{hp}")
                 for hp in range(2)]
            # wout (fp16, 2MB) loaded whole; needed from first out-proj on
            wout_t = wp.tile([KC, 2 * NDC * SB], f16, tag="wout")

            def proj(s):
                """QKV projection for seq block s (f32r)."""
                xs = xts[s]
                for m in range(4):
                    ps = pp.tile([KC, SB], f32, tag="accum", bufs=2,
                                 name=f"psqk{m}_{s}")
                    for k in range(NDC):
                        nc.tensor.matmul(
                            ps[:],
                            wqk_t[:, k * 4 * KC + m * KC:
                                  k * 4 * KC + (m + 1) * KC],
                            xs[:, k * SB:(k + 1) * SB],
                            start=(k == 0),
                            stop=(k == NDC - 1),
                        )
                        if s == 0 and m == 0 and k == 3:
                            # early wout kick: overlaps with projections
                            nc.gpsimd.dma_start(
                                out=wout_t[:].rearrange(
                                    "p (k c) -> p k c", k=2 * NDC),
                                in_=wout[:].rearrange(
                                    "n k p c -> p (n k) c"),
                            )
                    dest = (QQ if m < 2 else KK)[m % 2][s]
                    nc.vector.tensor_scalar_add(
                        dest[:], ps[:], bqk_t[:, m:m + 1])
                for sc in range(NCS):
                    pv = pp.tile([KC, 2 * KC], f32, tag="accum", bufs=2,
                                 name=f"psv{sc}_{s}")
                    for k in range(NDC):
                        nc.tensor.matmul(
                            pv[:],
                            xs[:, k * SB + sc * KC:k * SB + (sc + 1) * KC],
                            wv_t[:, k * 2 * KC:(k + 1) * 2 * KC],
                            start=(k == 0),
                            stop=(k == NDC - 1),
                        )
                    vv = V_all[s][:].rearrange(
                        "p (v k c) -> p v k c", v=4, k=NCS)
                    nc.vector.tensor_add(
                        vv[:, :, sc, 0:DH],
                        pv[:].rearrange("p (v c) -> p v c", c=DH),
                        bv_bc[:].rearrange("p (v c) -> p v c", c=DH),
                    )

            def attn(q):
                """Attention for query block q, both head pairs."""
                nkc = 4 * (q + 1)
                for hp in range(2):
                    P = []
                    for kc in range(nkc):
                        d = kc - 4 * q
                        c0 = KC * max(d, 0)
                        ps = pp.tile([KC, 2 * SB], f32, tag="pss", bufs=2,
                                     name=f"pss{hp}_{q}_{kc}")
                        for hh in range(2):
                            r0 = hh * DH
                            nc.tensor.matmul(
                                ps[:, hh * SB + c0:(hh + 1) * SB],
                                KK[hp][kc // 4][r0:r0 + DH,
                                                (kc % 4) * KC:
                                                (kc % 4 + 1) * KC],
                                QQ[hp][q][r0:r0 + DH, c0:SB],
                                start=True,
                                stop=True,
                            )
                        pt = pb.tile([KC, 2 * SB], f16, tag="P", bufs=8,
                                     name=f"P{hp}_{q}_{kc}")
                        P.append(pt)
                        ps3 = ps[:].rearrange("p (h f) -> p h f", h=2)
                        pd3 = pt[:].rearrange("p (h f) -> p h f", h=2)
                        nc.scalar.activation(
                            pd3[:, :, c0:SB],
                            ps3[:, :, c0:SB],
                            mybir.ActivationFunctionType.Exp,
                            scale=1.0 / float(np.sqrt(DH)),
                        )
                        if d >= 0:  # diagonal chunk: zero where k > q
                            nc.gpsimd.affine_select(
                                out=pd3[:, :, c0:SB],
                                in_=pd3[:, :, c0:SB],
                                pattern=[[0, 2], [1, SB - c0]],
                                compare_op=mybir.AluOpType.is_ge,
                                fill=0.0,
                                base=0,
                                channel_multiplier=-1,
                            )
                    pos = [pp.tile([DH + 1, SB], f32, tag="pos", bufs=2,
                                   name=f"pos{hh}_{hp}_{q}")
                           for hh in range(2)]
                    for kc in range(nkc):
                        d = kc - 4 * q
                        c0 = KC * max(d, 0)
                        for hh in range(2):
                            v = 2 * hp + hh
                            nc.tensor.matmul(
                                pos[hh][:, c0:SB],
                                V_all[kc // 4][:, v * NCS * (DH + 1) +
                                               (kc % 4) * (DH + 1):
                                               v * NCS * (DH + 1) +
                                               (kc % 4 + 1) * (DH + 1)],
                                P[kc][:, hh * SB + c0:(hh + 1) * SB],
                                start=(kc == 0),
                                stop=(kc == nkc - 1),
                            )
                    # normalize: O = pos[0:64] * (1 / pos[64]) per head
                    for hh in range(2):
                        den0 = ep.tile([1, SB], f32, tag=f"den{hh}", bufs=2,
                                       name=f"den{hh}_{hp}_{q}")
                        nc.vector.tensor_copy(den0[:], pos[hh][DH:DH + 1, :])
                        rden = ep.tile([1, SB], f32, tag=f"rden{hh}", bufs=2)
                        rscr = ep.tile([1, SB], f32, tag=f"rscr{hh}", bufs=2)
                        nc.vector.reciprocal_approx_accurate(
                            rden[:], den0[:], rscr[:])
                        rbc = ep.tile([DH, SB], f32, tag=f"rbc{hh}", bufs=2,
                                      name=f"rbc{hh}_{hp}_{q}")
                        nc.gpsimd.partition_broadcast(
                            out_ap=rbc[:],
                            in_ap=rden[:],
                        )
                        r0 = hh * DH
                        nc.vector.tensor_mul(
                            O[hp][r0:r0 + DH, q * SB:(q + 1) * SB],
                            pos[hh][0:DH, :],
                            rbc[:],
                        )
                    # stage this head-pair's quarter for the AllToAll (both
                    # group halves get a copy)
                    for g in range(2):
                        nc.gpsimd.dma_start(
                            out=a2a_in[q][4 * g:4 * (g + 1), hp].rearrange(
                                "t p c -> p t c"),
                            in_=O[hp][:, q * SB:(q + 1) * SB].rearrange(
                                "p (t c) -> p t c", t=4),
                        )

            def collective(q):
                nc.gpsimd.collective_compute(
                    "AllToAll",
                    mybir.AluOpType.bypass,
                    replica_groups=groups,
                    ins=[a2a_in[q][:]],
                    outs=[a2a_out[q][:]],
                )

            # my group's sender blocks start at slot 4*(rank//4)
            pid = nc.sync.partition_id()
            soff = (pid // 4) * 4

            def outproj(q):
                """Project the received 128-query chunk of quarter q."""
                recv = ep.tile([KC, NDC * KC], f16, tag="recv", bufs=2,
                               name=f"recv{q}")
                nc.sync.dma_start(
                    out=recv[:].rearrange("p (k c) -> p k c", k=NDC),
                    in_=a2a_out[q][bass.ds(soff, 4)].rearrange(
                        "s h p c -> p (s h) c"),
                )
                for nb in range(2):
                    po = pp.tile([KC, SB], f32, tag="accum", bufs=2,
                                 name=f"po{nb}_{q}")
                    for k in range(NDC):
                        nc.tensor.matmul(
                            po[:],
                            recv[:, k * KC:(k + 1) * KC],
                            wout_t[:, (nb * NDC + k) * SB:
                                   (nb * NDC + k + 1) * SB],
                            start=(k == 0),
                            stop=(k == NDC - 1),
                        )
                    ot = ep.tile([KC, SB], f32, tag="osb", bufs=2,
                                 name=f"ot{nb}_{q}")
                    nc.vector.tensor_add(
                        ot[:], po[:], bo_bc[:, nb * SB:(nb + 1) * SB])
                    nc.sync.dma_start(
                        out=out_ext[q, :, nb * SB:(nb + 1) * SB],
                        in_=ot[:],
                    )

            # ---- pipeline -------------------------------------------------
            proj(0)
            nc.gpsimd.dma_start(
                out=xts[2][:].rearrange("p (k c) -> p k c", k=NDC),
                in_=xt[2].rearrange("k p c -> p k c"),
            )
            proj(1)
            nc.gpsimd.dma_start(
                out=xts[3][:].rearrange("p (k c) -> p k c", k=NDC),
                in_=xt[3].rearrange("k p c -> p k c"),
            )
            attn(0)
            collective(0)
            proj(2)
            attn(1)
            collective(1)
            proj(3)
            outproj(0)
            attn(2)
            collective(2)
            outproj(1)
            attn(3)
            collective(3)
            outproj(2)
            outproj(3)

    nc.compile()
    return nc


def _get_program():
    global _compiled
    if _compiled is None:
        _compiled = _build()
    return _compiled


def _shard_inputs(x, Wqkv, bqkv, Wout, bout):
    """Build the 8 per-core input maps (all host-side numpy)."""
    x = np.ascontiguousarray(x, dtype=np.float32)
    Wqkv = np.asarray(Wqkv, dtype=np.float32)
    bqkv = np.asarray(bqkv, dtype=np.float32)
    Wout = np.asarray(Wout, dtype=np.float32)
    bout = np.ascontiguousarray(np.asarray(bout, dtype=np.float32))

    Wq = Wqkv[:, 0 * D:1 * D]
    Wk = Wqkv[:, 1 * D:2 * D]
    Wv_full = Wqkv[:, 2 * D:3 * D]
    bq = bqkv[0 * D:1 * D]
    bk = bqkv[1 * D:2 * D]
    bv_full = bqkv[2 * D:3 * D]

    # per batch: [NSB, NDC, KC, SB] blocked transpose of x
    xts = []
    for b in range(B):
        xts.append(np.ascontiguousarray(
            x[b].T                                   # [D, S]
            .reshape(NDC, KC, NSB, SB).transpose(2, 0, 1, 3)
        ))
    wout_b = np.ascontiguousarray(
        Wout.reshape(NDC, KC, 2, SB).transpose(2, 0, 1, 3)
        .astype(np.float16))
    bo_row = np.ascontiguousarray(bout.reshape(1, D))
    vones = np.ones((KC, 16), dtype=np.float16)

    in_maps = []
    for c in range(NCORES):
        b, r = c // 4, c % 4
        h0 = 4 * r
        cols = lambda W, i: W[:, (h0 + i) * DH:(h0 + i + 2) * DH]
        wqk_c = np.ascontiguousarray(np.concatenate(
            [cols(Wq, 0), cols(Wq, 2), cols(Wk, 0), cols(Wk, 2)],
            axis=1).reshape(NDC, KC, 4 * KC))
        bqk_c = np.ascontiguousarray(np.stack(
            [bq[(h0) * DH:(h0 + 2) * DH], bq[(h0 + 2) * DH:(h0 + 4) * DH],
             bk[(h0) * DH:(h0 + 2) * DH], bk[(h0 + 2) * DH:(h0 + 4) * DH]],
            axis=1))
        wv_c = np.ascontiguousarray(
            Wv_full[:, h0 * DH:(h0 + 4) * DH].reshape(NDC, KC, 2 * KC))
        bv_c = np.ascontiguousarray(
            bv_full[h0 * DH:(h0 + 4) * DH].reshape(1, 2 * KC))
        in_maps.append({
            "xt": xts[b], "wqk": wqk_c, "wv": wv_c, "wout": wout_b,
            "bqk": bqk_c, "bv": bv_c, "bo": bo_row, "vones": vones,
        })
    return in_maps


def run(inputs, trace=False, trace_kwargs=None):
    nc = _get_program()
    in_maps = _shard_inputs(**inputs)
    res = run_bass_kernel_spmd(
        nc, in_maps, list(range(NCORES)), trace=trace,
        **(trace_kwargs or {}),
    )
    out = np.empty((B, S, D), dtype=np.float32)
    for c in range(NCORES):
        b, r = c // 4, c % 4
        o = res.results[c]["out"]          # [NSB, KC, D]
        for q in range(NSB):
            out[b, SB * q + KC * r:SB * q + KC * (r + 1), :] = o[q]
    return out, res


def kernel(**inputs):
    out, _ = run(inputs)
    return out
